# revision 1
# baseline (speedup 1.0000x reference)
"""Trainium2 Bass kernel for DeepGCN (nn_DeepGCN_82454782148693).

Strategy (8 NeuronCores, SPMD single program):
  - Nodes sharded contiguously: core k owns rows [k*12500, (k+1)*12500),
    locally padded to 12544 = 98*128. Feature-major resident state
    xT [128, 12544] in SBUF.
  - Per layer: h' = (x @ conv_W) * dinv computed locally (PE matmuls +
    PE transposes), written node-major to DRAM, AllGather -> full
    h'_all [100352, 128] replicated per core.
  - Sparse aggregation (pull, dst-sharded): hardware indirect gather
    (dma_gather, int16 indices over 4 banks of 25088 rows) pulls source
    rows into SBUF one-row-per-partition; segment-sum is done on the
    TensorEngine as one-hot matmuls: psum[dst_tile] += S^T @ msgs where
    S[e, d] = (local_dst[e] == d) is generated on the fly by the vector
    engine (iota + is_equal).  Self-loops are applied separately as an
    identity matmul on the locally owned h' rows (keeps the gather
    schedule padding small).
  - BatchNorm: per-feature sums/sumsq accumulated via accum_out, tiny
    AllReduce, affine folded into one scale/shift pair.
  - Residuals + relu on DVE, classifier on PE, output written
    feature-major [2, 12544] per core and untransposed on host.

Host-side work is limited to sharding/layout: integer edge bucketing and
sorting, padding, and array transposes. All floating-point math runs on
device.
"""

import os
import sys

import numpy as np

for _p in ("/opt/trn_rl_repo", "/root/.axon_site/_ro/trn_rl_repo"):
    if os.path.isdir(_p) and _p not in sys.path:
        sys.path.append(_p)

import concourse.bass as bass
import concourse.bacc as bacc
import concourse.mybir as mybir
import concourse.tile as tile
from concourse import bass_utils

F32 = mybir.dt.float32
I16 = mybir.dt.int16
AF = mybir.ActivationFunctionType
OP = mybir.AluOpType
AX = mybir.AxisListType


class Cfg:
    def __init__(self, N=100000, E=1600000, NCORES=8, GT=7):
        self.N, self.E, self.NCORES = N, E, NCORES
        self.H, self.L, self.HC, self.C = 128, 4, 64, 2
        self.ALPHA, self.THETA, self.EPS = 0.1, 0.5, 1e-5
        assert N % NCORES == 0
        self.NSH = N // NCORES                    # real nodes per core
        self.TILES = (self.NSH + 127) // 128
        self.NPAD = self.TILES * 128
        self.NP = NCORES * self.NPAD              # padded global rows
        self.BANKS = 4
        assert self.NP % self.BANKS == 0
        self.BROWS = self.NP // self.BANKS
        assert self.BROWS <= 32768, "int16 gather index range"
        self.GT = GT                              # dst tiles per group
        assert self.TILES % GT == 0
        self.NG = self.TILES // GT
        self.GSUB = 8                             # max chunks per dma_gather call
        # node chunks of <=512 for dense phases
        self.PCH = []
        off = 0
        while off < self.NPAD:
            w = min(512, self.NPAD - off)
            self.PCH.append((off, w))
            off += w
        # lin parts covering GT*128
        self.LPARTS = []
        off = 0
        while off < GT * 128:
            w = min(512, GT * 128 - off)
            self.LPARTS.append((off, w))
            off += w


CFG = Cfg()


# ----------------------------------------------------------------------------
# Host preprocessing: edge bucketing into the static gather schedule.
# ----------------------------------------------------------------------------

def build_host_data(edge_index, cfg):
    """Returns (deg_layout [NCORES,128,TILES], idx arrays, dst arrays, K_b)."""
    c = cfg
    src = edge_index[0].astype(np.int64)
    dst = edge_index[1].astype(np.int64)

    # degree including self-loops (reference semantics)
    deg = np.bincount(dst, minlength=c.N).astype(np.float32) + 1.0

    core = dst // c.NSH
    ldst = dst - core * c.NSH
    tile_id = ldst >> 7
    dloc = ldst & 127
    spad = (src // c.NSH) * c.NPAD + (src % c.NSH)
    bank = spad // c.BROWS
    bidx = (spad - bank * c.BROWS).astype(np.int64)

    counts = np.zeros((c.NCORES, c.TILES, c.BANKS), dtype=np.int64)
    np.add.at(counts, (core, tile_id, bank), 1)
    K_b = [max(1, int(np.ceil(counts[:, :, b].max() / 128))) for b in range(c.BANKS)]

    # slot layout: per core, groups-major, then bank, then tile-in-group
    boff = np.cumsum([0] + [c.GT * K_b[b] * 128 for b in range(c.BANKS)])
    SLOTG = int(boff[-1])                         # slots per group
    TOT = c.NG * SLOTG                            # slots per core
    idx_slots = np.zeros((c.NCORES, TOT), dtype=np.int16)
    dst_slots = np.full((c.NCORES, TOT), 300.0, dtype=np.float32)

    order = np.lexsort((bank, tile_id, core))
    c_s, t_s, b_s = core[order], tile_id[order], bank[order]
    bi_s, dl_s = bidx[order], dloc[order]
    key = (c_s * c.TILES + t_s) * c.BANKS + b_s
    runstart = np.r_[0, np.flatnonzero(np.diff(key)) + 1]
    runid = np.zeros(len(key), dtype=np.int64)
    runid[runstart[1:]] = 1
    runid = np.cumsum(runid)
    pos = np.arange(len(key)) - runstart[runid]

    g_s, tp_s = t_s // c.GT, t_s % c.GT
    Kb_arr = np.array(K_b, dtype=np.int64)
    base = g_s * SLOTG + boff[b_s] + tp_s * Kb_arr[b_s] * 128
    slots = base + pos
    idx_slots[c_s, slots] = bi_s.astype(np.int16)
    dst_slots[c_s, slots] = dl_s.astype(np.float32)

    # idx array wrapped in 16 partitions, replicated to 128
    assert TOT % 16 == 0
    idx_wrapped = idx_slots.reshape(c.NCORES, TOT // 16, 16).transpose(0, 2, 1)
    idx_in = np.tile(idx_wrapped, (1, 8, 1)).astype(np.int16)  # [NCORES,128,TOT//16]

    # dst array: [128, NCHUNK] per core; chunk j holds slots [j*128,(j+1)*128)
    NCHUNK = TOT // 128
    dst_in = dst_slots.reshape(c.NCORES, NCHUNK, 128).transpose(0, 2, 1).copy()

    # degree layout [NCORES, 128, TILES]: [p, t] = deg of local node t*128+p
    deg_in = np.ones((c.NCORES, c.NPAD), dtype=np.float32)
    deg_in[:, :c.NSH] = deg.reshape(c.NCORES, c.NSH)
    deg_in = deg_in.reshape(c.NCORES, c.TILES, 128).transpose(0, 2, 1).copy()

    return deg_in, idx_in, dst_in, K_b, SLOTG, [int(x) for x in boff]


def pack_weights(inputs, cfg):
    """Pack all weights into [128, WCOLS] and biases into [128, NB]."""
    c = cfg
    H, L, HC, Cc = c.H, c.L, c.HC, c.C
    cols = []
    cols.append(np.asarray(inputs["proj_W"], np.float32))            # [128,128]
    for l in range(L):
        cols.append(np.asarray(inputs["conv_W"][l], np.float32))
    for l in range(L):
        cols.append(np.asarray(inputs["lin_W"][l], np.float32))
    cols.append(np.asarray(inputs["cls_W1"], np.float32))            # [128, 64]
    w2 = np.zeros((H, Cc), np.float32)
    w2[:HC] = np.asarray(inputs["cls_W2"], np.float32)               # [64,2] pad
    cols.append(w2)
    W = np.concatenate(cols, axis=1)                                 # [128, ...]

    nb = np.zeros((H, 19), np.float32)
    nb[:, 0] = np.asarray(inputs["proj_b"], np.float32)
    for l in range(L):
        nb[:, 1 + l] = np.asarray(inputs["conv_b"][l], np.float32)
        nb[:, 5 + l] = np.asarray(inputs["lin_b"][l], np.float32)
        nb[:, 9 + l] = np.asarray(inputs["bn_g"][l], np.float32)
        nb[:, 13 + l] = np.asarray(inputs["bn_b"][l], np.float32)
    nb[:HC, 17] = np.asarray(inputs["cls_b1"], np.float32)
    nb[:Cc, 18] = np.asarray(inputs["cls_b2"], np.float32)
    return W, nb


# ----------------------------------------------------------------------------
# Device program
# ----------------------------------------------------------------------------

def build_program(cfg, K_b, SLOTG, boff):
    c = cfg
    H, L = c.H, c.L
    NCHUNK = c.NG * SLOTG // 128
    IDXCOLS = c.NG * SLOTG // 16
    GCOLS = SLOTG // 16
    WCOLS = 128 * (1 + 2 * L) + c.HC + c.C
    C1 = float(1.0 - c.ALPHA - c.THETA)

    nc = bacc.Bacc(
        "TRN2",
        target_bir_lowering=False,
        debug=False,
        enable_asserts=False,
        num_devices=c.NCORES,
    )

    # ---- I/O ----
    xT_in = nc.dram_tensor("xT_in", [H, c.NPAD], F32, kind="ExternalInput").ap()
    deg_in = nc.dram_tensor("deg_in", [H, c.TILES], F32, kind="ExternalInput").ap()
    idx_in = nc.dram_tensor("idx_in", [H, IDXCOLS], I16, kind="ExternalInput").ap()
    dst_in = nc.dram_tensor("dst_in", [H, NCHUNK], F32, kind="ExternalInput").ap()
    w_in = nc.dram_tensor("w_in", [H, WCOLS], F32, kind="ExternalInput").ap()
    b_in = nc.dram_tensor("b_in", [H, 19], F32, kind="ExternalInput").ap()
    out_d = nc.dram_tensor("out_d", [c.C, c.NPAD], F32, kind="ExternalOutput").ap()

    # ---- internal DRAM ----
    hsh_d = nc.dram_tensor("hsh_d", [c.NPAD, H], F32, kind="Internal").ap()
    hall_d = nc.dram_tensor(
        "hall_d", [c.NP, H], F32, kind="Internal", addr_space="Shared"
    ).ap()
    h2_d = nc.dram_tensor("h2_d", [H, c.NPAD], F32, kind="Internal").ap()
    stin_d = nc.dram_tensor("stin_d", [H, 2], F32, kind="Internal").ap()
    stout_d = nc.dram_tensor(
        "stout_d", [H, 2], F32, kind="Internal", addr_space="Shared"
    ).ap()

    # ---- static SBUF residents ----
    xT = nc.alloc_sbuf_tensor("xT", [H, c.NPAD], F32).ap()
    x0s = nc.alloc_sbuf_tensor("x0s", [H, c.NPAD], F32).ap()
    dstv = nc.alloc_sbuf_tensor("dstv", [H, NCHUNK], F32).ap()
    dinv = nc.alloc_sbuf_tensor("dinv", [H, c.TILES], F32).ap()
    iota = nc.alloc_sbuf_tensor("iota", [H, H], F32).ap()
    ident = nc.alloc_sbuf_tensor("ident", [H, H], F32).ap()
    zeros = nc.alloc_sbuf_tensor("zeros", [H, 512], F32).ap()
    wsb = nc.alloc_sbuf_tensor("wsb", [H, WCOLS], F32).ap()
    bsb = nc.alloc_sbuf_tensor("bsb", [H, 19], F32).ap()
    sums = nc.alloc_sbuf_tensor("sums", [H, 64], F32).ap()
    sqs = nc.alloc_sbuf_tensor("sqs", [H, 64], F32).ap()
    stat = nc.alloc_sbuf_tensor("stat", [H, 12], F32).ap()

    wproj = wsb[:, 0:128]
    wconv = lambda l: wsb[:, 128 * (1 + l):128 * (2 + l)]
    wlin = lambda l: wsb[:, 128 * (1 + L + l):128 * (2 + L + l)]
    wcls1 = wsb[:, 128 * (1 + 2 * L):128 * (1 + 2 * L) + c.HC]
    wcls2 = wsb[:c.HC, 128 * (1 + 2 * L) + c.HC:WCOLS]

    rg = [list(range(c.NCORES))]

    hall_banks = [hall_d[b * c.BROWS:(b + 1) * c.BROWS, :] for b in range(c.BANKS)]

    with tile.TileContext(nc) as tc:
        # ================= P0: prologue =================
        with tc.sbuf_pool(name="p0", bufs=3) as pool, \
             tc.psum_pool(name="p0p", bufs=2) as pp:
            nc.sync.dma_start(wsb, w_in)
            nc.sync.dma_start(bsb, b_in)
            nc.sync.dma_start(dstv, dst_in)
            degs = pool.tile([H, c.TILES], F32)
            nc.sync.dma_start(degs, deg_in)
            # dinv = 1/sqrt(deg)
            rec = pool.tile([H, c.TILES], F32)
            nc.vector.reciprocal(rec, degs)
            nc.scalar.sqrt(dinv, rec)
            # iota (value = free index), partition-index, identity
            nc.gpsimd.iota(iota, pattern=[[1, H]], base=0, channel_multiplier=0,
                           allow_small_or_imprecise_dtypes=True)
            pidx = pool.tile([H, H], F32)
            nc.gpsimd.iota(pidx, pattern=[[0, H]], base=0, channel_multiplier=1,
                           allow_small_or_imprecise_dtypes=True)
            nc.vector.tensor_tensor(ident, iota, pidx, OP.is_equal)
            nc.vector.memset(zeros, 0.0)
            # proj + relu -> xT ; x0s = alpha * xT
            for (off, w) in c.PCH:
                xin = pool.tile([H, 512], F32, tag="xin")
                nc.sync.dma_start(xin[:, :w], xT_in[:, off:off + w])
                ps = pp.tile([H, 512], F32, tag="ps")
                nc.tensor.matmul(ps[:, :w], wproj, xin[:, :w])
                nc.scalar.activation(xT[:, off:off + w], ps[:, :w], AF.Relu,
                                     bias=bsb[:, 0:1], scale=1.0)
                nc.vector.tensor_scalar_mul(x0s[:, off:off + w], xT[:, off:off + w],
                                            c.ALPHA)
            if c.NSH < c.NPAD:
                nc.vector.memset(xT[:, c.NSH:c.NPAD], 0.0)
                nc.vector.memset(x0s[:, c.NSH:c.NPAD], 0.0)

        # ================= layers =================
        for li in range(L):
            # ---- P1: h' = (x @ convW) * dinv, node-major -> hsh_d ----
            with tc.sbuf_pool(name=f"l{li}a", bufs=3) as pool, \
                 tc.psum_pool(name=f"l{li}ap", bufs=2) as pp, \
                 tc.psum_pool(name=f"l{li}at", bufs=2) as pt:
                for (off, w) in c.PCH:
                    ps = pp.tile([H, 512], F32, tag="ps")
                    nc.tensor.matmul(ps[:, :w], wconv(li), xT[:, off:off + w])
                    hT = pool.tile([H, 512], F32, tag="hT")
                    nc.vector.tensor_copy(hT[:, :w], ps[:, :w])
                    stg = pool.tile([H, 512], F32, tag="stg")
                    for j in range(w // 128):
                        t = off // 128 + j
                        tp2 = pt.tile([H, H], F32, tag="tp2")
                        nc.tensor.transpose(tp2, hT[:, j * 128:(j + 1) * 128], ident)
                        nc.vector.tensor_scalar_mul(stg[:, j * 128:(j + 1) * 128],
                                                    tp2, dinv[:, t:t + 1])
                    dram = hsh_d[off:off + w, :].rearrange("(j p) f -> p j f", p=128)
                    nc.sync.dma_start(
                        dram, stg[:, :w].rearrange("p (j f) -> p j f", f=H))

            # ---- P2: AllGather h' ----
            nc.gpsimd.collective_compute(
                "AllGather", OP.bypass, replica_groups=rg,
                ins=[hsh_d], outs=[hall_d])

            # ---- P3: gather + one-hot segment-sum + lin + stats ----
            with tc.sbuf_pool(name=f"l{li}g", bufs=2) as pool, \
                 tc.sbuf_pool(name=f"l{li}s", bufs=4) as spool, \
                 tc.psum_pool(name=f"l{li}ga", bufs=2) as ppa, \
                 tc.psum_pool(name=f"l{li}gt", bufs=2) as ppt, \
                 tc.psum_pool(name=f"l{li}gl", bufs=2) as ppl:
                nA = min(c.GT, 4)
                nB = c.GT - nA
                sc = 0
                for g in range(c.NG):
                    idxt = pool.tile([H, GCOLS], I16, tag="idxt")
                    nc.sync.dma_start(idxt, idx_in[:, g * GCOLS:(g + 1) * GCOLS])
                    hloc = pool.tile([H, c.GT * 128], F32, tag="hloc")
                    r0 = g * c.GT * 128
                    nc.sync.dma_start(
                        hloc.rearrange("p (j f) -> p j f", f=H),
                        hsh_d[r0:r0 + c.GT * 128, :].rearrange(
                            "(j p) f -> p j f", p=128))
                    aggA = ppa.tile([H, 512], F32, tag="aggA", name="aggA")
                    aggB = (ppa.tile([H, 512], F32, tag="aggB", name="aggB")
                            if nB else None)
                    # zero accumulation banks, then self-loop identity matmuls
                    nc.tensor.matmul(aggA[:, :nA * 128], ident, zeros[:, :nA * 128],
                                     start=True, stop=False, skip_group_check=True)
                    if nB:
                        nc.tensor.matmul(aggB[:, :nB * 128], ident,
                                         zeros[:, :nB * 128],
                                         start=True, stop=False,
                                         skip_group_check=True)

                    def agg_slice(tp):
                        if tp < nA:
                            return aggA[:, tp * 128:(tp + 1) * 128]
                        return aggB[:, (tp - nA) * 128:(tp - nA + 1) * 128]

                    for tp in range(c.GT):
                        nc.tensor.matmul(agg_slice(tp), ident,
                                         hloc[:, tp * 128:(tp + 1) * 128],
                                         start=False, stop=False,
                                         skip_group_check=True)
                    for b in range(c.BANKS):
                        CH = c.GT * K_b[b]
                        nidx = CH * 128
                        msgs = pool.tile([H, c.GT * max(K_b) * 128], F32, tag="msgs")
                        for j0 in range(0, CH, c.GSUB):
                            j1 = min(j0 + c.GSUB, CH)
                            nsub = (j1 - j0) * 128
                            col0 = (boff[b] + j0 * 128) // 16
                            nc.gpsimd.dma_gather(
                                out_ap=msgs[:, j0 * 128:j1 * 128].rearrange(
                                    "p (ch f) -> p ch f", f=H),
                                in_ap=hall_banks[b],
                                idxs_ap=idxt[:, col0:col0 + nsub // 16],
                                num_idxs=nsub,
                                num_idxs_reg=nsub,
                                elem_size=H,
                            )
                        colbase = (g * SLOTG + boff[b]) // 128
                        for j in range(CH):
                            tp = j // K_b[b]
                            S = spool.tile([H, H], F32, tag="S")
                            nc.vector.tensor_scalar(
                                S, iota, dstv[:, colbase + j:colbase + j + 1], None,
                                op0=OP.is_equal)
                            last = (b == c.BANKS - 1) and (j == CH - 1)
                            nc.tensor.matmul(
                                agg_slice(tp), S, msgs[:, j * 128:(j + 1) * 128],
                                start=False, stop=last, skip_group_check=True)
                    # finalize tiles: y = agg*dinv -> transpose -> +conv_b
                    yT = pool.tile([H, c.GT * 128], F32, tag="yT")
                    for tp in range(c.GT):
                        t = g * c.GT + tp
                        y = spool.tile([H, H], F32, tag="y")
                        nc.vector.tensor_scalar_mul(y, agg_slice(tp),
                                                    dinv[:, t:t + 1])
                        yp = ppt.tile([H, H], F32, tag="yp")
                        nc.tensor.transpose(yp, y, ident)
                        nc.scalar.activation(yT[:, tp * 128:(tp + 1) * 128], yp,
                                             AF.Identity, bias=bsb[:, 1 + li:2 + li],
                                             scale=1.0)
                    # lin + bias + stats, spill h2 to DRAM
                    for (off, w) in c.LPARTS:
                        ps3 = ppl.tile([H, 512], F32, tag="ps3")
                        nc.tensor.matmul(ps3[:, :w], wlin(li), yT[:, off:off + w])
                        h2t = pool.tile([H, 512], F32, tag="h2t")
                        gcol = g * c.GT * 128 + off
                        is_pad_part = gcol + w > c.NSH
                        if not is_pad_part:
                            nc.vector.tensor_scalar(
                                h2t[:, :w], ps3[:, :w], bsb[:, 5 + li:6 + li], None,
                                op0=OP.add, op1=OP.add,
                                accum_out=sums[:, sc:sc + 1])
                        else:
                            nc.vector.tensor_scalar(
                                h2t[:, :w], ps3[:, :w], bsb[:, 5 + li:6 + li], None,
                                op0=OP.add)
                            nc.vector.memset(h2t[:, c.NSH - gcol:w], 0.0)
                            nc.vector.tensor_reduce(
                                sums[:, sc:sc + 1], h2t[:, :w], AX.X, OP.add)
                        sq = pool.tile([H, 512], F32, tag="sq")
                        nc.vector.scalar_tensor_tensor(
                            sq[:, :w], h2t[:, :w], 0.0, h2t[:, :w],
                            op0=OP.add, op1=OP.mult, accum_out=sqs[:, sc:sc + 1])
                        sc += 1
                        nc.sync.dma_start(h2_d[:, gcol:gcol + w], h2t[:, :w])
                nparts = sc

            # ---- P4: stats allreduce + scale/shift ----
            with tc.sbuf_pool(name=f"l{li}r", bufs=2) as pool:
                nc.vector.tensor_reduce(stat[:, 0:1], sums[:, :nparts], AX.X, OP.add)
                nc.vector.tensor_reduce(stat[:, 1:2], sqs[:, :nparts], AX.X, OP.add)
                nc.sync.dma_start(stin_d, stat[:, 0:2])
                nc.gpsimd.collective_compute(
                    "AllReduce", OP.add, replica_groups=rg,
                    ins=[stin_d], outs=[stout_d])
                nc.sync.dma_start(stat[:, 2:4], stout_d)
                invn = 1.0 / float(c.N)
                # mean = S1/N ; var = S2/N - mean^2 ; inv = 1/sqrt(var+eps)
                nc.vector.tensor_scalar_mul(stat[:, 4:5], stat[:, 2:3], invn)
                m2 = pool.tile([H, 1], F32)
                nc.vector.tensor_tensor(m2, stat[:, 4:5], stat[:, 4:5], OP.mult)
                nc.vector.scalar_tensor_tensor(stat[:, 5:6], stat[:, 3:4], invn, m2,
                                               op0=OP.mult, op1=OP.subtract)
                vps = pool.tile([H, 1], F32)
                nc.vector.tensor_scalar_add(vps, stat[:, 5:6], float(c.EPS))
                sd = pool.tile([H, 1], F32)
                nc.scalar.sqrt(sd, vps)
                inv = pool.tile([H, 1], F32)
                nc.vector.reciprocal(inv, sd)
                # s = C1 * g * inv ; u = C1*b - mean*s
                gi = pool.tile([H, 1], F32)
                nc.vector.tensor_tensor(gi, inv, bsb[:, 9 + li:10 + li], OP.mult)
                nc.vector.tensor_scalar_mul(stat[:, 6:7], gi, C1)
                ms = pool.tile([H, 1], F32)
                nc.vector.tensor_tensor(ms, stat[:, 4:5], stat[:, 6:7], OP.mult)
                nc.vector.scalar_tensor_tensor(stat[:, 7:8], bsb[:, 13 + li:14 + li],
                                               C1, ms, op0=OP.mult, op1=OP.subtract)

            # ---- P5: x = relu(s*h2 + u + alpha*x0 + theta*x_prev) ----
            with tc.sbuf_pool(name=f"l{li}f", bufs=3) as pool:
                for (off, w) in c.PCH:
                    h2c = pool.tile([H, 512], F32, tag="h2c")
                    nc.sync.dma_start(h2c[:, :w], h2_d[:, off:off + w])
                    t1 = pool.tile([H, 512], F32, tag="t1")
                    nc.vector.tensor_scalar(t1[:, :w], h2c[:, :w], stat[:, 6:7],
                                            stat[:, 7:8], op0=OP.mult, op1=OP.add)
                    t2 = pool.tile([H, 512], F32, tag="t2")
                    nc.vector.scalar_tensor_tensor(t2[:, :w], xT[:, off:off + w],
                                                   float(c.THETA), t1[:, :w],
                                                   op0=OP.mult, op1=OP.add)
                    t3 = pool.tile([H, 512], F32, tag="t3")
                    nc.vector.tensor_tensor(t3[:, :w], t2[:, :w],
                                            x0s[:, off:off + w], OP.add)
                    nc.vector.tensor_scalar_max(xT[:, off:off + w], t3[:, :w], 0.0)
                if c.NSH < c.NPAD:
                    nc.vector.memset(xT[:, c.NSH:c.NPAD], 0.0)

        # ================= P6: classifier =================
        with tc.sbuf_pool(name="p6", bufs=3) as pool, \
             tc.psum_pool(name="p6p", bufs=2) as pp, \
             tc.psum_pool(name="p6q", bufs=2) as pq:
            for (off, w) in c.PCH:
                ps = pp.tile([c.HC, 512], F32, tag="ps")
                nc.tensor.matmul(ps[:, :w], wcls1, xT[:, off:off + w])
                h3 = pool.tile([c.HC, 512], F32, tag="h3")
                nc.scalar.activation(h3[:, :w], ps[:, :w], AF.Relu,
                                     bias=bsb[:c.HC, 17:18], scale=1.0)
                ps2 = pq.tile([c.C, 512], F32, tag="ps2")
                nc.tensor.matmul(ps2[:, :w], wcls2, h3[:, :w])
                ot = pool.tile([c.C, 512], F32, tag="ot")
                nc.vector.tensor_scalar(ot[:, :w], ps2[:, :w],
                                        bsb[:c.C, 18:19], None, op0=OP.add)
                nc.sync.dma_start(out_d[:, off:off + w], ot[:, :w])

    nc.compile()
    return nc


# ----------------------------------------------------------------------------
# Full pipeline
# ----------------------------------------------------------------------------

LAST_RESULTS = None
_PROGRAM_CACHE = {}


def make_in_maps(inputs, cfg):
    c = cfg
    x = np.ascontiguousarray(np.asarray(inputs["x"], np.float32))
    edge_index = np.asarray(inputs["edge_index"])
    deg_in, idx_in, dst_in, K_b, SLOTG, boff = build_host_data(edge_index, c)
    W, B = pack_weights(inputs, c)

    in_maps = []
    for k in range(c.NCORES):
        xs = np.zeros((c.H, c.NPAD), np.float32)
        xs[:, :c.NSH] = x[k * c.NSH:(k + 1) * c.NSH].T
        in_maps.append({
            "xT_in": xs,
            "deg_in": np.ascontiguousarray(deg_in[k]),
            "idx_in": np.ascontiguousarray(idx_in[k]),
            "dst_in": np.ascontiguousarray(dst_in[k]),
            "w_in": W,
            "b_in": B,
        })
    return in_maps, K_b, SLOTG, boff


def kernel(**inputs):
    global LAST_RESULTS
    c = CFG
    in_maps, K_b, SLOTG, boff = make_in_maps(inputs, c)

    key = (tuple(K_b), SLOTG)
    if key not in _PROGRAM_CACHE:
        _PROGRAM_CACHE[key] = build_program(c, K_b, SLOTG, boff)
    nc = _PROGRAM_CACHE[key]

    trace = bool(int(os.environ.get("GCN_TRACE", "0")))
    res = bass_utils.run_bass_kernel_spmd(
        nc, in_maps, core_ids=list(range(c.NCORES)), trace=trace)
    LAST_RESULTS = res

    out = np.empty((c.N, c.C), np.float32)
    for k in range(c.NCORES):
        o = res.results[k]["out_d"]           # [C, NPAD]
        out[k * c.NSH:(k + 1) * c.NSH] = o[:, :c.NSH].T
    return out



# revision 2
# speedup vs baseline: 1.0409x; 1.0409x over previous
"""Trainium2 Bass kernel v2 for DeepGCN (nn_DeepGCN_82454782148693).

8-core SPMD, dst-sharded. fp16 data path (validated: rel err ~3e-3 in sim).
Per layer:
  P1: hT = conv(xT) (fp16 matmul, f32 psum); PE-transpose per tile; scale by
      dinv_src per-partition; shard write fp16 -> hsh_d.
  AG: AllGather fp16 -> hall_d [100352, 128] (doubles as the barrier).
  P3: slots = edges+self-loops sorted by (bank, tile), dense (bucket sizes =
      max over cores, SPMD-common schedule; ~7% pad). dma_gather (int16,
      4 banks, 1024-idx calls) pulls msg rows; host-built S fp16 (dinv_dst
      folded; boundary chunks get one S per overlapped tile) streams from
      HBM; PE: aggT[f,dst-tile] += msgs_chunk^T-free matmul(lhsT=msgs,
      rhs=S). Lin matmul per 2-tile window; h2 fp16 resident; BN stats via
      accum_out. conv_b/lin_b dropped (absorbed exactly by BN).
  P4: tiny stats AllReduce; fold BN affine + C1.
  P5: xT = relu(s*h2 + u + x0s + theta*xT); zero pad cols.
Classifier feature-major, f32 out [2, 12544]/core.
"""

import os
import sys

import numpy as np

for _p in ("/opt/trn_rl_repo", "/root/.axon_site/_ro/trn_rl_repo"):
    if os.path.isdir(_p) and _p not in sys.path:
        sys.path.append(_p)

import concourse.bass as bass
import concourse.bacc as bacc
import concourse.mybir as mybir
import concourse.tile as tile
from concourse import bass_utils

F32 = mybir.dt.float32
F16 = mybir.dt.float16
I16 = mybir.dt.int16
AF = mybir.ActivationFunctionType
OP = mybir.AluOpType
AX = mybir.AxisListType

N = 100000
NCORES = 8
NSH = N // NCORES
TILES = 98
NPAD = TILES * 128
NP = NCORES * NPAD
BANKS = 4
BROWS = NP // BANKS
H = 128
L, HC, C = 4, 64, 2
ALPHA, THETA, EPS = 0.1, 0.5, 1e-5
C1 = float(1.0 - ALPHA - THETA)
WTILES = 2
GSUB = 8          # chunks per dma_gather call


# ----------------------------------------------------------------------------
# Host preprocessing (SPMD-common schedule)
# ----------------------------------------------------------------------------

def host_prep(edge_index):
    src0 = np.asarray(edge_index[0], np.int64)
    dst0 = np.asarray(edge_index[1], np.int64)
    loops = np.arange(N, dtype=np.int64)
    src = np.concatenate([src0, loops])
    dst = np.concatenate([dst0, loops])

    deg = np.bincount(dst0, minlength=N).astype(np.float32) + 1.0
    dinv = (1.0 / np.sqrt(deg)).astype(np.float32)

    core = dst // NSH
    tile_id = (dst - core * NSH) >> 7
    part = (dst - core * NSH) & 127
    spad = (src // NSH) * NPAD + (src % NSH)
    bank = spad // BROWS
    bidx = (spad - bank * BROWS).astype(np.int64)
    sval = dinv[dst]

    # counts per (core, bank, tile)
    cnt = np.zeros((NCORES, BANKS, TILES), np.int64)
    np.add.at(cnt, (core, bank, tile_id), 1)
    nmax = cnt.max(axis=0)                      # [BANKS, TILES]

    # common slot layout: bank stream = concat_t nmax[b, t]; bank end padded
    # to a multiple of 128.
    bucket0 = np.zeros((BANKS, TILES + 1), np.int64)   # slot offset in bank
    bank_slots = np.zeros(BANKS, np.int64)
    for b in range(BANKS):
        bucket0[b, 1:] = np.cumsum(nmax[b])
        bank_slots[b] = ((bucket0[b, -1] + 127) // 128) * 128
    bank_c = bank_slots // 128                  # chunks per bank
    bank_chunk0 = np.concatenate([[0], np.cumsum(bank_c)])
    Ctot = int(bank_c.sum())
    TOT = Ctot * 128
    bank_slot0 = bank_chunk0 * 128              # global slot offset per bank

    # common (chunk, tile) schedule, chunk-major si enumeration
    sched = []        # (global_chunk, tile, si, bank)
    si = 0
    tile_nmm = np.zeros(TILES, np.int64)
    for b in range(BANKS):
        for ch in range(int(bank_c[b])):
            s0, s1 = ch * 128, (ch + 1) * 128
            t0 = int(np.searchsorted(bucket0[b], s0, side="right") - 1)
            t1 = int(np.searchsorted(bucket0[b], s1 - 1, side="right") - 1)
            t1 = min(t1, TILES - 1)
            for t in range(t0, t1 + 1):
                if bucket0[b, t + 1] > s0 and bucket0[b, t] < s1 \
                        and nmax[b, t] > 0:
                    sched.append((int(bank_chunk0[b] + ch), t, si, b))
                    tile_nmm[t] += 1
                    si += 1
    NS = si

    # per-core slot fill
    order = np.argsort(((core * BANKS + bank) * TILES + tile_id), kind="stable")
    co, bo, to_, po, io, vo = (core[order], bank[order], tile_id[order],
                               part[order], bidx[order], sval[order])
    # position within (core, bank, tile)
    keyo = ((co * BANKS + bo) * TILES + to_)
    runstart = np.r_[0, np.flatnonzero(np.diff(keyo)) + 1]
    runid = np.zeros(len(keyo), np.int64)
    runid[runstart[1:]] = 1
    runid = np.cumsum(runid)
    pos = np.arange(len(keyo)) - runstart[runid]
    slot_global = bank_slot0[bo] + bucket0[bo, to_] + pos

    cores = []
    for c in range(NCORES):
        sel = co == c
        sl = slot_global[sel]
        idx = np.zeros(TOT, np.int16)
        idx[sl] = io[sel].astype(np.int16)
        Sarr = np.zeros((128, NS * 128), np.float16)
        # column of S: need si for (chunk, tile) pair
        ch_of = sl >> 7
        si_map = {}
        for (chg, t, s, b) in sched:
            si_map[(chg, t)] = s
        si_of = np.array([si_map[(int(chv), int(tv))]
                          for chv, tv in zip(ch_of, to_[sel])], np.int64)
        Sarr[sl & 127, (si_of << 7) + po[sel]] = vo[sel]

        idx16 = idx.reshape(TOT // 16, 16).T
        idx16 = np.tile(idx16, (8, 1)).astype(np.int16)
        cores.append(dict(idx16=np.ascontiguousarray(idx16),
                          S=np.ascontiguousarray(Sarr)))

    meta = dict(Ctot=Ctot, NS=NS, sched=sched, tile_nmm=tile_nmm,
                bank_c=bank_c.astype(np.int64),
                bank_chunk0=bank_chunk0.astype(np.int64))
    return dinv, meta, cores


def pack_weights(inputs):
    cols = [np.asarray(inputs["proj_W"], np.float32)]
    for l in range(L):
        cols.append(np.asarray(inputs["conv_W"][l], np.float32))
    for l in range(L):
        cols.append(np.asarray(inputs["lin_W"][l], np.float32))
    cols.append(np.asarray(inputs["cls_W1"], np.float32))
    w2 = np.zeros((H, C), np.float32)
    w2[:HC] = np.asarray(inputs["cls_W2"], np.float32)
    cols.append(w2)
    W = np.concatenate(cols, axis=1).astype(np.float16)
    nb = np.zeros((H, 7), np.float32)
    nb[:, 0] = np.asarray(inputs["proj_b"], np.float32)
    nb[:HC, 5] = np.asarray(inputs["cls_b1"], np.float32)
    nb[:C, 6] = np.asarray(inputs["cls_b2"], np.float32)
    bn = np.zeros((H, 2 * L), np.float32)
    for l in range(L):
        bn[:, l] = np.asarray(inputs["bn_g"][l], np.float32)
        bn[:, L + l] = np.asarray(inputs["bn_b"][l], np.float32)
    return W, nb, bn


# ----------------------------------------------------------------------------
# Device program
# ----------------------------------------------------------------------------

def build_program(meta):
    Ctot, NS = meta["Ctot"], meta["NS"]
    sched = meta["sched"]
    tile_nmm = meta["tile_nmm"]
    bank_c = meta["bank_c"]
    bank_chunk0 = meta["bank_chunk0"]
    WCOLS = H * (1 + 2 * L) + HC + C
    IDXCOLS = Ctot * 128 // 16

    nc = bacc.Bacc("TRN2", target_bir_lowering=False, debug=False,
                   enable_asserts=False, num_devices=NCORES)

    xT_in = nc.dram_tensor("xT_in", [H, NPAD], F16, kind="ExternalInput").ap()
    dinv_in = nc.dram_tensor("dinv_in", [H, TILES], F32,
                             kind="ExternalInput").ap()
    idx_in = nc.dram_tensor("idx_in", [H, IDXCOLS], I16,
                            kind="ExternalInput").ap()
    s_in = nc.dram_tensor("s_in", [H, NS * H], F16, kind="ExternalInput").ap()
    w_in = nc.dram_tensor("w_in", [H, WCOLS], F16, kind="ExternalInput").ap()
    b_in = nc.dram_tensor("b_in", [H, 7], F32, kind="ExternalInput").ap()
    bn_in = nc.dram_tensor("bn_in", [H, 2 * L], F32, kind="ExternalInput").ap()
    out_d = nc.dram_tensor("out_d", [C, NPAD], F32, kind="ExternalOutput").ap()

    hsh_d = nc.dram_tensor("hsh_d", [NPAD, H], F16, kind="Internal").ap()
    hall_d = nc.dram_tensor("hall_d", [NP, H], F16, kind="Internal",
                            addr_space="Shared").ap()
    stin_d = nc.dram_tensor("stin_d", [H, 2], F32, kind="Internal").ap()
    stout_d = nc.dram_tensor("stout_d", [H, 2], F32, kind="Internal",
                             addr_space="Shared").ap()

    xT = nc.alloc_sbuf_tensor("xT", [H, NPAD], F16).ap()
    x0s = nc.alloc_sbuf_tensor("x0s", [H, NPAD], F16).ap()
    h2 = nc.alloc_sbuf_tensor("h2", [H, NPAD], F16).ap()
    idxs = nc.alloc_sbuf_tensor("idxs", [H, IDXCOLS], I16).ap()
    wsb = nc.alloc_sbuf_tensor("wsb", [H, WCOLS], F16).ap()
    bsb = nc.alloc_sbuf_tensor("bsb", [H, 7], F32).ap()
    bnsb = nc.alloc_sbuf_tensor("bnsb", [H, 2 * L], F32).ap()
    dinv = nc.alloc_sbuf_tensor("dinv", [H, TILES], F32).ap()
    ident = nc.alloc_sbuf_tensor("ident", [H, H], F16).ap()
    sums = nc.alloc_sbuf_tensor("sums", [H, 64], F32).ap()
    sqs = nc.alloc_sbuf_tensor("sqs", [H, 64], F32).ap()
    stat = nc.alloc_sbuf_tensor("stat", [H, 12], F32).ap()

    wproj = wsb[:, 0:H]
    wconv = lambda l: wsb[:, H * (1 + l):H * (2 + l)]
    wlin = lambda l: wsb[:, H * (1 + L + l):H * (2 + L + l)]
    wcls1 = wsb[:, H * (1 + 2 * L):H * (1 + 2 * L) + HC]
    wcls2 = wsb[:HC, H * (1 + 2 * L) + HC:WCOLS]

    rg = [list(range(NCORES))]
    hall_banks = [hall_d[b * BROWS:(b + 1) * BROWS, :] for b in range(BANKS)]
    PCH = [(o, min(512, NPAD - o)) for o in range(0, NPAD, 512)]

    # window structure: WTILES tiles each; per (window, bank): si range and
    # matmul list
    windows = []
    for t0 in range(0, TILES, WTILES):
        t1 = min(t0 + WTILES, TILES)
        per_bank = []
        for b in range(BANKS):
            mms = [e for e in sched if e[3] == b and t0 <= e[1] < t1]
            per_bank.append(mms)
        windows.append((t0, t1, per_bank))

    with tile.TileContext(nc) as tc:
        # ================= P0 =================
        with tc.sbuf_pool(name="p0", bufs=3) as pool, \
             tc.psum_pool(name="p0p", bufs=2) as pp:
            nc.sync.dma_start(wsb, w_in)
            nc.sync.dma_start(bsb, b_in)
            nc.sync.dma_start(bnsb, bn_in)
            nc.sync.dma_start(dinv, dinv_in)
            nc.sync.dma_start(idxs, idx_in)
            iota = pool.tile([H, H], F32, tag="iota")
            pidx = pool.tile([H, H], F32, tag="pidx")
            nc.gpsimd.iota(iota, pattern=[[1, H]], base=0, channel_multiplier=0,
                           allow_small_or_imprecise_dtypes=True)
            nc.gpsimd.iota(pidx, pattern=[[0, H]], base=0, channel_multiplier=1,
                           allow_small_or_imprecise_dtypes=True)
            nc.vector.tensor_tensor(ident, iota, pidx, OP.is_equal)
            for (off, w) in PCH:
                xin = pool.tile([H, 512], F16, tag="xin")
                nc.sync.dma_start(xin[:, :w], xT_in[:, off:off + w])
                ps = pp.tile([H, 512], F32, tag="ps")
                nc.tensor.matmul(ps[:, :w], wproj, xin[:, :w])
                nc.scalar.activation(xT[:, off:off + w], ps[:, :w], AF.Relu,
                                     bias=bsb[:, 0:1], scale=1.0)
                nc.vector.tensor_scalar_mul(x0s[:, off:off + w],
                                            xT[:, off:off + w], ALPHA)
            if NSH < NPAD:
                nc.vector.memset(xT[:, NSH:NPAD], 0.0)
                nc.vector.memset(x0s[:, NSH:NPAD], 0.0)

        for li in range(L):
            # ---- P1 ----
            with tc.sbuf_pool(name=f"l{li}a", bufs=3) as pool, \
                 tc.psum_pool(name=f"l{li}ap", bufs=2) as pp, \
                 tc.psum_pool(name=f"l{li}at", bufs=2) as pt:
                for (off, w) in PCH:
                    ps = pp.tile([H, 512], F32, tag="ps")
                    nc.tensor.matmul(ps[:, :w], wconv(li), xT[:, off:off + w])
                    hT = pool.tile([H, 512], F16, tag="hT")
                    nc.vector.tensor_copy(hT[:, :w], ps[:, :w])
                    stg = pool.tile([H, 512], F16, tag="stg")
                    for j in range(w // 128):
                        t = off // 128 + j
                        tp2 = pt.tile([H, H], F16, tag="tp2")
                        nc.tensor.transpose(tp2, hT[:, j * 128:(j + 1) * 128],
                                            ident)
                        nc.vector.tensor_scalar_mul(
                            stg[:, j * 128:(j + 1) * 128], tp2,
                            dinv[:, t:t + 1])
                    dram = hsh_d[off:off + w, :].rearrange(
                        "(j p) f -> p j f", p=128)
                    nc.sync.dma_start(
                        dram, stg[:, :w].rearrange("p (j f) -> p j f", f=H))

            # ---- AG ----
            nc.gpsimd.collective_compute(
                "AllGather", OP.bypass, replica_groups=rg,
                ins=[hsh_d], outs=[hall_d])

            # ---- P3 ----
            with tc.sbuf_pool(name=f"l{li}g", bufs=5) as gpool, \
                 tc.sbuf_pool(name=f"l{li}s", bufs=2) as spool, \
                 tc.sbuf_pool(name=f"l{li}h", bufs=3) as hpool, \
                 tc.psum_pool(name=f"l{li}pa", bufs=5) as ppa, \
                 tc.psum_pool(name=f"l{li}pl", bufs=2) as ppl:
                # per-bank sequential gather calls feeding a ring of tiles
                msgs_of = {}       # global chunk -> (tile_handle, local_off)
                next_call = [int(bank_chunk0[b]) for b in range(BANKS)]

                def ensure_gathered(chg, b):
                    while chg >= next_call[b]:
                        c0 = next_call[b]
                        cn = min(GSUB, int(bank_chunk0[b + 1]) - c0)
                        mt = gpool.tile([H, GSUB * H], F16, tag=f"m{b}")
                        nc.gpsimd.dma_gather(
                            out_ap=mt[:, :cn * H].rearrange(
                                "p (c f) -> p c f", f=H),
                            in_ap=hall_banks[b],
                            idxs_ap=idxs[:, c0 * 8:(c0 + cn) * 8],
                            num_idxs=cn * 128, num_idxs_reg=cn * 128,
                            elem_size=H)
                        for k in range(cn):
                            msgs_of[c0 + k] = (mt, k)
                        next_call[b] = c0 + cn

                tile_done = np.zeros(TILES, np.int64)
                psum_of = {}
                sc = 0
                for (t0, t1, per_bank) in windows:
                    # S tiles per bank
                    for b in range(BANKS):
                        mms = per_bank[b]
                        if not mms:
                            continue
                        si0, si1 = mms[0][2], mms[-1][2] + 1
                        st = spool.tile([H, 16 * H], F16, tag=f"s{b}")
                        ns = si1 - si0
                        assert ns <= 16, (t0, b, ns)
                        nc.sync.dma_start(st[:, :ns * H],
                                          s_in[:, si0 * H:si1 * H])
                        for (chg, t, si, _b) in mms:
                            ensure_gathered(chg, b)
                            mt, lo = msgs_of[chg]
                            if t not in psum_of:
                                psum_of[t] = ppa.tile([H, H], F32, tag="agg",
                                                      name="agg")
                            first = tile_done[t] == 0
                            last = tile_done[t] == tile_nmm[t] - 1
                            nc.tensor.matmul(
                                psum_of[t],
                                mt[:, lo * H:(lo + 1) * H],
                                st[:, (si - si0) * H:(si - si0 + 1) * H],
                                start=bool(first), stop=bool(last),
                                skip_group_check=True)
                            tile_done[t] += 1
                    # finalize closed tiles of this window
                    nwc = (t1 - t0) * H
                    aggs = hpool.tile([H, WTILES * H], F16, tag="aggs")
                    for t in range(t0, t1):
                        if t in psum_of:
                            nc.vector.tensor_copy(
                                aggs[:, (t - t0) * H:(t - t0 + 1) * H],
                                psum_of.pop(t))
                        else:
                            nc.vector.memset(
                                aggs[:, (t - t0) * H:(t - t0 + 1) * H], 0.0)
                    ps3 = ppl.tile([H, WTILES * H], F32, tag="ps3")
                    nc.tensor.matmul(ps3[:, :nwc], wlin(li), aggs[:, :nwc])
                    gcol = t0 * H
                    nc.vector.tensor_scalar(
                        h2[:, gcol:gcol + nwc], ps3[:, :nwc], 0.0, None,
                        op0=OP.add, op1=OP.add,
                        accum_out=sums[:, sc:sc + 1])
                    sq = hpool.tile([H, WTILES * H], F16, tag="sq")
                    nc.vector.scalar_tensor_tensor(
                        sq[:, :nwc], h2[:, gcol:gcol + nwc], 0.0,
                        h2[:, gcol:gcol + nwc],
                        op0=OP.add, op1=OP.mult, accum_out=sqs[:, sc:sc + 1])
                    sc += 1
                    if sc > 64:
                        raise RuntimeError("stats overflow")
                nparts = sc

            # ---- P4 ----
            with tc.sbuf_pool(name=f"l{li}r", bufs=2) as pool:
                nc.vector.tensor_reduce(stat[:, 0:1], sums[:, :nparts], AX.X,
                                        OP.add)
                nc.vector.tensor_reduce(stat[:, 1:2], sqs[:, :nparts], AX.X,
                                        OP.add)
                nc.sync.dma_start(stin_d, stat[:, 0:2])
                nc.gpsimd.collective_compute(
                    "AllReduce", OP.add, replica_groups=rg,
                    ins=[stin_d], outs=[stout_d])
                nc.sync.dma_start(stat[:, 2:4], stout_d)
                invn = 1.0 / float(N)
                nc.vector.tensor_scalar_mul(stat[:, 4:5], stat[:, 2:3], invn)
                m2 = pool.tile([H, 1], F32)
                nc.vector.tensor_tensor(m2, stat[:, 4:5], stat[:, 4:5],
                                        OP.mult)
                nc.vector.scalar_tensor_tensor(stat[:, 5:6], stat[:, 3:4],
                                               invn, m2, op0=OP.mult,
                                               op1=OP.subtract)
                vps = pool.tile([H, 1], F32)
                nc.vector.tensor_scalar_add(vps, stat[:, 5:6], float(EPS))
                sd = pool.tile([H, 1], F32)
                nc.scalar.sqrt(sd, vps)
                inv = pool.tile([H, 1], F32)
                nc.vector.reciprocal(inv, sd)
                gi = pool.tile([H, 1], F32)
                nc.vector.tensor_tensor(gi, inv, bnsb[:, li:li + 1], OP.mult)
                nc.vector.tensor_scalar_mul(stat[:, 6:7], gi, C1)
                ms = pool.tile([H, 1], F32)
                nc.vector.tensor_tensor(ms, stat[:, 4:5], stat[:, 6:7],
                                        OP.mult)
                nc.vector.scalar_tensor_tensor(
                    stat[:, 7:8], bnsb[:, L + li:L + li + 1], C1, ms,
                    op0=OP.mult, op1=OP.subtract)

            # ---- P5 ----
            with tc.sbuf_pool(name=f"l{li}f", bufs=3) as pool:
                for (off, w) in PCH:
                    t1_ = pool.tile([H, 512], F32, tag="t1")
                    nc.vector.tensor_scalar(t1_[:, :w], h2[:, off:off + w],
                                            stat[:, 6:7], stat[:, 7:8],
                                            op0=OP.mult, op1=OP.add)
                    t2 = pool.tile([H, 512], F32, tag="t2")
                    nc.vector.scalar_tensor_tensor(
                        t2[:, :w], xT[:, off:off + w], THETA, t1_[:, :w],
                        op0=OP.mult, op1=OP.add)
                    t3 = pool.tile([H, 512], F32, tag="t3")
                    nc.vector.tensor_tensor(t3[:, :w], t2[:, :w],
                                            x0s[:, off:off + w], OP.add)
                    nc.vector.tensor_scalar_max(xT[:, off:off + w],
                                                t3[:, :w], 0.0)
                if NSH < NPAD:
                    nc.vector.memset(xT[:, NSH:NPAD], 0.0)

        # ================= P6 =================
        with tc.sbuf_pool(name="p6", bufs=3) as pool, \
             tc.psum_pool(name="p6p", bufs=2) as pp, \
             tc.psum_pool(name="p6q", bufs=2) as pq:
            for (off, w) in PCH:
                ps = pp.tile([HC, 512], F32, tag="ps")
                nc.tensor.matmul(ps[:, :w], wcls1, xT[:, off:off + w])
                h3 = pool.tile([HC, 512], F16, tag="h3")
                nc.scalar.activation(h3[:, :w], ps[:, :w], AF.Relu,
                                     bias=bsb[:HC, 5:6], scale=1.0)
                ps2 = pq.tile([C, 512], F32, tag="ps2")
                nc.tensor.matmul(ps2[:, :w], wcls2, h3[:, :w])
                ot = pool.tile([C, 512], F32, tag="ot")
                nc.vector.tensor_scalar(ot[:, :w], ps2[:, :w],
                                        bsb[:C, 6:7], None, op0=OP.add)
                nc.sync.dma_start(out_d[:, off:off + w], ot[:, :w])

    nc.compile()
    return nc


# ----------------------------------------------------------------------------
# Orchestration
# ----------------------------------------------------------------------------

LAST_RESULTS = None
_PROGRAM_CACHE = {}


def kernel(**inputs):
    global LAST_RESULTS
    edge_index = np.asarray(inputs["edge_index"])
    dinv, meta, cores = host_prep(edge_index)
    W, NB, BN = pack_weights(inputs)

    x = np.asarray(inputs["x"], np.float32)
    dl = np.ones(NCORES * NPAD, np.float32)
    for c in range(NCORES):
        dl[c * NPAD:c * NPAD + NSH] = dinv[c * NSH:(c + 1) * NSH]
    dinv_nm = dl.reshape(NCORES, TILES, 128).transpose(0, 2, 1)

    in_maps = []
    for c in range(NCORES):
        xs = np.zeros((H, NPAD), np.float16)
        xs[:, :NSH] = x[c * NSH:(c + 1) * NSH].T.astype(np.float16)
        in_maps.append({
            "xT_in": xs,
            "dinv_in": np.ascontiguousarray(dinv_nm[c]),
            "idx_in": cores[c]["idx16"],
            "s_in": cores[c]["S"],
            "w_in": W,
            "b_in": NB,
            "bn_in": BN,
        })

    key = (meta["Ctot"], meta["NS"])
    if key not in _PROGRAM_CACHE:
        _PROGRAM_CACHE[key] = build_program(meta)
    nc = _PROGRAM_CACHE[key]

    trace = bool(int(os.environ.get("GCN_TRACE", "0")))
    res = bass_utils.run_bass_kernel_spmd(
        nc, in_maps, core_ids=list(range(NCORES)), trace=trace)
    LAST_RESULTS = res

    out = np.empty((N, C), np.float32)
    for c in range(NCORES):
        o = res.results[c]["out_d"]
        out[c * NSH:(c + 1) * NSH] = o[:, :NSH].T
    return out
